# revision 1
# baseline (speedup 1.0000x reference)
"""Trainium2 Bass kernel for nn_Criterion (retrieval_knn, B=4 V=8192 F=16384 N=8192).

Per-core work (8 cores, data-parallel over B with 2-way split of N per batch):
  phase 0: gather face vertices (indirect DMA), compute centers / unit normals /
           plane offsets; store rhs table caug[4,F] and gather table T[F,4] in DRAM.
  phase 1: per 128-point chunk: PE matmul (K=4 augmented -> score = |c|^2 - 2 p.c)
           into PSUM, DVE prefix-min scan over all F scores, ACT Sign+accum count
           pass recovers the exact first-occurrence argmin index.
  finish:  indirect-gather (n, c.n) per point, distance = p.n - c.n,
           interp = relu(eps - d); outputs per-partition partial sums.
Host sums the 8x128x2 partials into (loss, perc).
"""

import numpy as np

B, V, F, N = 4, 8192, 16384, 8192
NCORES = 8
SPLIT = 2                 # cores per batch
NS = N // SPLIT           # 4096 points per core
P = 128
CHUNKS = NS // P          # 32
FT = 2048                 # faces per scan block (4 PSUM banks)
NBLK = F // FT            # 8
FP = F // P               # 128 faces per partition in phase 0
EPS = 1e-3
WEIGHT = 1000.0
BIG = 3.0e38

_CACHE = {}


def _get_minscan_op():
    """Register (once) a fused custom-DVE op: out[k] = min(C0, in0[0..k]).

    Single-stream inclusive prefix-min with a per-partition carry-in scalar.
    Runs at ~1 elem/cycle fp32 vs ~2.2 cycles/elem for the stock
    TENSOR_TENSOR_SCAN (two input streams)."""
    from concourse import dve_ops as D
    from concourse.dve_spec import Spec, Src0, C0, scan, AluOp, lower

    for op in D.OPS:
        if op.name == "ANT_MIN_SCAN_CARRY":
            return op
    spec = Spec(body=scan(AluOp.MIN, Src0, init=C0))
    op = D.DveOp("ANT_MIN_SCAN_CARRY", spec, subdim=False, uops_sha={})
    D.OPS.append(op)
    D._SUB_OPCODE_FOR_NAME[op.name] = D._CUSTOM_DVE_ROW_BASE + len(D.OPS) - 1
    D.CUSTOM_DVE_SPECS[op.name] = spec
    for ver in ("v3", "v4"):
        s = D.DveOpSpec(name=op.name, opcode=D.get_dve_sub_opcode(op.name),
                        uops=lower(spec, ver=ver), rd1_en=D.has_src1(spec))
        op.uops_sha[ver] = s.sha(ver)
    return op


def _build_nc(use_f32r=True, reps=1, stage='full'):
    import concourse.mybir as mybir
    import concourse.tile as tile
    import concourse.bass as bass
    from concourse import bacc

    f32 = mybir.dt.float32
    f32r = mybir.dt.float32r
    i32 = mybir.dt.int32
    Alu = mybir.AluOpType
    Act = mybir.ActivationFunctionType
    X = mybir.AxisListType.X

    nc = bacc.Bacc(None, target_bir_lowering=False)

    pred = nc.dram_tensor("pred", [NS, 3], f32, kind="ExternalInput")
    predT = nc.dram_tensor("predT", [3, NS], f32, kind="ExternalInput")
    opos = nc.dram_tensor("opos", [V, 3], f32, kind="ExternalInput")
    faces = nc.dram_tensor("faces", [F, 3], i32, kind="ExternalInput")
    out = nc.dram_tensor("out", [P, 2], f32, kind="ExternalOutput")
    tdram = nc.dram_tensor("tdram", [F, 4], f32, kind="Internal")
    caug = nc.dram_tensor("caug", [24, F], mybir.dt.bfloat16, kind="Internal")

    i16 = mybir.dt.int16
    # vertex-gather scratch in DRAM, one per corner; rows are in "slot order"
    # l = c*2048 + j  <->  face (16c + j%16)*128 + j//16.  All face tables
    # (caug columns, tdram rows) use this same internal order; outputs are
    # order-agnostic reductions so the permutation never leaks.
    vscr = [nc.dram_tensor(f"vscr{k}", [F, 3], f32, kind="Internal") for k in range(3)]

    with tile.TileContext(nc) as tc:
        # ---------------- phase 0a: gather face vertices via ap_gather ----------------
        with tc.tile_pool(name="g0", bufs=1) as g0:
            faces_sb = g0.tile([P, FP, 3], i32)
            nc.sync.dma_start(
                out=faces_sb[:], in_=faces[:].rearrange("(p i) k -> p i k", p=P)
            )
            fpl16 = g0.tile([P, 3, FP], i16)
            for k in range(3):
                nc.vector.tensor_copy(out=fpl16[:, k, :], in_=faces_sb[:, :, k])
            vtxrep = g0.tile([P, V, 3], f32)
            vtx_bcast = bass.AP(opos[:].tensor, 0, [[0, P], [3, V], [1, 3]])
            nc.sync.dma_start(out=vtxrep[:], in_=vtx_bcast)
            NIDX = 16 * FP  # 2048 indices per Q7 core
            for k in range(3):
                vg = g0.tile([P, NIDX, 3], f32, name=f"vg{k}", tag=f"vg{k}")
                nc.gpsimd.ap_gather(
                    vg[:], vtxrep[:], fpl16[:, k, :],
                    channels=P, num_elems=V, d=3, num_idxs=NIDX,
                )
                # keep one partition per Q7 core: slot order l = c*2048 + j
                for c in range(8):
                    nc.sync.dma_start(
                        out=vscr[k][c * NIDX:(c + 1) * NIDX, :],
                        in_=vg[16 * c:16 * c + 1, :, :],
                    )

        # ---------------- phase 0b: face table math ----------------
        with tc.tile_pool(name="ph0", bufs=1) as ph0:
            v = []
            for k in range(3):
                vk = ph0.tile([P, FP, 3], f32, name=f"v{k}")
                nc.sync.dma_start(
                    out=vk[:], in_=vscr[k][:].rearrange("(p i) m -> p i m", p=P)
                )
                v.append(vk)
            # centers
            cc = ph0.tile([P, FP, 3], f32)
            nc.vector.tensor_tensor(out=cc[:], in0=v[0][:], in1=v[1][:], op=Alu.add)
            nc.vector.tensor_tensor(out=cc[:], in0=cc[:], in1=v[2][:], op=Alu.add)
            nc.vector.tensor_scalar_mul(cc[:], cc[:], 1.0 / 3.0)
            # edges
            e1 = ph0.tile([P, FP, 3], f32)
            e2 = ph0.tile([P, FP, 3], f32)
            nc.vector.tensor_tensor(out=e1[:], in0=v[1][:], in1=v[0][:], op=Alu.subtract)
            nc.vector.tensor_tensor(out=e2[:], in0=v[2][:], in1=v[0][:], op=Alu.subtract)
            # cross product -> tint[:, :, 0:3]; plane offset b -> tint[:, :, 3]
            tint = ph0.tile([P, FP, 4], f32)
            tmp = ph0.tile([P, FP], f32)
            tmp2 = ph0.tile([P, FP], f32)
            for j in range(3):
                a, b2 = (j + 1) % 3, (j + 2) % 3
                nc.vector.tensor_tensor(out=tmp[:], in0=e1[:, :, a], in1=e2[:, :, b2], op=Alu.mult)
                nc.vector.tensor_tensor(out=tmp2[:], in0=e1[:, :, b2], in1=e2[:, :, a], op=Alu.mult)
                nc.vector.tensor_tensor(out=tint[:, :, j], in0=tmp[:], in1=tmp2[:], op=Alu.subtract)
            # norm
            nn2 = ph0.tile([P, FP], f32)
            nc.vector.tensor_tensor(out=nn2[:], in0=tint[:, :, 0], in1=tint[:, :, 0], op=Alu.mult)
            for j in (1, 2):
                nc.vector.tensor_tensor(out=tmp[:], in0=tint[:, :, j], in1=tint[:, :, j], op=Alu.mult)
                nc.vector.tensor_tensor(out=nn2[:], in0=nn2[:], in1=tmp[:], op=Alu.add)
            nc.scalar.sqrt(tmp[:], nn2[:])
            nc.vector.tensor_scalar_max(tmp[:], tmp[:], 1e-12)
            nc.vector.reciprocal(tmp2[:], tmp[:])
            for j in range(3):
                nc.vector.tensor_tensor(out=tint[:, :, j], in0=tint[:, :, j], in1=tmp2[:], op=Alu.mult)
            # b = c . n
            nc.vector.tensor_tensor(out=tmp[:], in0=cc[:, :, 0], in1=tint[:, :, 0], op=Alu.mult)
            nc.vector.tensor_tensor(out=tmp2[:], in0=cc[:, :, 1], in1=tint[:, :, 1], op=Alu.mult)
            nc.vector.tensor_tensor(out=tmp[:], in0=tmp[:], in1=tmp2[:], op=Alu.add)
            nc.vector.tensor_tensor(out=tmp2[:], in0=cc[:, :, 2], in1=tint[:, :, 2], op=Alu.mult)
            nc.vector.tensor_tensor(out=tint[:, :, 3], in0=tmp[:], in1=tmp2[:], op=Alu.add)
            # caug planar: rows c0,c1,c2,|c|^2  stored [P, 4, FP]
            cpl = ph0.tile([P, 4, FP], f32)
            for j in range(3):
                nc.vector.tensor_copy(out=cpl[:, j, :], in_=cc[:, :, j])
            nc.vector.tensor_tensor(out=cpl[:, 3, :], in0=cc[:, :, 0], in1=cc[:, :, 0], op=Alu.mult)
            for j in (1, 2):
                nc.vector.tensor_tensor(out=tmp[:], in0=cc[:, :, j], in1=cc[:, :, j], op=Alu.mult)
                nc.vector.tensor_tensor(out=cpl[:, 3, :], in0=cpl[:, 3, :], in1=tmp[:], op=Alu.add)
            bf = mybir.dt.bfloat16
            ch16 = ph0.tile([P, 4, FP], bf)
            nc.vector.tensor_copy(out=ch16[:], in_=cpl[:])
            chf = ph0.tile([P, 4, FP], f32)
            nc.vector.tensor_copy(out=chf[:], in_=ch16[:])
            cr1 = ph0.tile([P, 4, FP], f32)
            nc.vector.tensor_tensor(out=cr1[:], in0=cpl[:], in1=chf[:], op=Alu.subtract)
            cm16 = ph0.tile([P, 4, FP], bf)
            nc.vector.tensor_copy(out=cm16[:], in_=cr1[:])
            cmf = ph0.tile([P, 4, FP], f32)
            nc.vector.tensor_copy(out=cmf[:], in_=cm16[:])
            cr2 = ph0.tile([P, 4, FP], f32)
            nc.vector.tensor_tensor(out=cr2[:], in0=cr1[:], in1=cmf[:], op=Alu.subtract)
            cl16 = ph0.tile([P, 4, FP], bf)
            nc.vector.tensor_copy(out=cl16[:], in_=cr2[:])
            # rows: [bh, bm, bh, bl, bh, bm] paired with lhs [ah, ah, am, ah, al, am]
            cpl24 = ph0.tile([P, 24, FP], bf)
            for i, part in enumerate((ch16, cm16, ch16, cl16, ch16, cm16)):
                nc.vector.tensor_copy(out=cpl24[:, 4 * i:4 * i + 4, :], in_=part[:])
            nc.sync.dma_start(
                out=tdram[:].rearrange("(p i) k -> p i k", p=P), in_=tint[:]
            )
            nc.sync.dma_start(
                out=caug[:].rearrange("j (p i) -> p j i", p=P), in_=cpl24[:]
            )

        # ---------------- phase 1: scores + argmin ----------------
        bf = mybir.dt.bfloat16
        with tc.tile_pool(name="const1", bufs=1) as constp:
            paug = constp.tile([24, NS], bf)
            with tc.tile_pool(name="ptmp", bufs=1) as ptmp:
                paug0 = ptmp.tile([4, NS], f32)
                nc.vector.memset(paug0[:], 1.0)
                nc.sync.dma_start(out=paug0[0:3, :], in_=predT[:])
                nc.scalar.mul(paug0[0:3, :], paug0[0:3, :], -2.0)
                ph16 = ptmp.tile([4, NS], bf)
                nc.vector.tensor_copy(out=ph16[:], in_=paug0[:])
                phf = ptmp.tile([4, NS], f32)
                nc.vector.tensor_copy(out=phf[:], in_=ph16[:])
                pr1 = ptmp.tile([4, NS], f32)
                nc.vector.tensor_tensor(out=pr1[:], in0=paug0[:], in1=phf[:], op=Alu.subtract)
                pm16 = ptmp.tile([4, NS], bf)
                nc.vector.tensor_copy(out=pm16[:], in_=pr1[:])
                pmf = ptmp.tile([4, NS], f32)
                nc.vector.tensor_copy(out=pmf[:], in_=pm16[:])
                pr2 = ptmp.tile([4, NS], f32)
                nc.vector.tensor_tensor(out=pr2[:], in0=pr1[:], in1=pmf[:], op=Alu.subtract)
                pl16 = ptmp.tile([4, NS], bf)
                nc.vector.tensor_copy(out=pl16[:], in_=pr2[:])
                # lhs rows: [ah, ah, am, ah, al, am]
                for i, part in enumerate((ph16, ph16, pm16, ph16, pl16, pm16)):
                    nc.sync.dma_start(out=paug[4 * i:4 * i + 4, :], in_=part[:])
            minscan = _get_minscan_op()
            caug_sb = constp.tile([24, F], bf)
            nc.sync.dma_start(out=caug_sb[:], in_=caug[:])
            pred_pts = constp.tile([P, CHUNKS, 3], f32)
            nc.sync.dma_start(
                out=pred_pts[:], in_=pred[:].rearrange("(k p) j -> p k j", p=P)
            )
            idx_all = constp.tile([P, CHUNKS], i32)
            g = constp.tile([P, CHUNKS, 4], f32)

            with (
                tc.tile_pool(name="psump", bufs=2, space="PSUM") as psump,
                tc.tile_pool(name="prefp", bufs=2) as prefp,
                tc.tile_pool(name="smallp", bufs=3) as smallp,
            ):
                for _rep in range(reps):
                  for k in range(CHUNKS):
                      prefix = prefp.tile([P, F], f32, tag="prefix")
                      lhsT = paug[:, k * P:(k + 1) * P]
                      for t in range(NBLK):
                          ps = psump.tile([P, FT], f32, tag="ps")
                          if stage not in ('nomm',):
                              for j in range(FT // 512):
                                  rr = caug_sb[:, t * FT + j * 512:t * FT + (j + 1) * 512]
                                  nc.tensor.matmul(
                                      ps[:, j * 512:(j + 1) * 512],
                                      lhsT,
                                      rr,
                                      start=True,
                                      stop=True,
                                  )
                          else:
                              nc.vector.memset(ps[:, 0:1], 0.0)
                          if stage not in ('noscan',):
                              init = BIG if t == 0 else prefix[:, t * FT - 1:t * FT]
                              nc.vector._custom_dve(
                                  minscan,
                                  out=prefix[:, t * FT:(t + 1) * FT],
                                  in0=ps[:],
                                  s0=init,
                              )
                      if stage in ('full', 'nogather'):
                          negm = smallp.tile([P, 1], f32, tag="negm")
                          nc.vector.tensor_scalar_mul(negm[:], prefix[:, F - 1:F], -1.0)
                          acc = smallp.tile([P, 1], f32, tag="acc")
                          nc.scalar.activation(
                              out=prefix[:],
                              in_=prefix[:],
                              func=Act.Sign,
                              bias=negm[:, 0:1],
                              scale=1.0,
                              accum_out=acc[:],
                          )
                          nc.vector.tensor_copy(out=idx_all[:, k:k + 1], in_=acc[:])
                      else:
                          nc.vector.memset(idx_all[:, k:k + 1], 0)
                      if stage != 'nogather':
                          nc.gpsimd.indirect_dma_start(
                              out=g[:, k, :],
                              out_offset=None,
                              in_=tdram[:],
                              in_offset=bass.IndirectOffsetOnAxis(ap=idx_all[:, k:k + 1], axis=0),
                          )

            # ---------------- finish ----------------
            with tc.tile_pool(name="finp", bufs=1) as finp:
                prod = finp.tile([P, CHUNKS, 3], f32)
                nc.vector.tensor_tensor(out=prod[:], in0=g[:, :, 0:3], in1=pred_pts[:], op=Alu.mult)
                s3 = finp.tile([P, CHUNKS], f32)
                nc.vector.tensor_reduce(out=s3[:], in_=prod[:], axis=X, op=Alu.add)
                d = finp.tile([P, CHUNKS], f32)
                nc.vector.tensor_tensor(out=d[:], in0=s3[:], in1=g[:, :, 3], op=Alu.subtract)
                interp = finp.tile([P, CHUNKS], f32)
                eps1 = finp.tile([P, 1], f32)
                nc.vector.memset(eps1[:], EPS)
                nc.scalar.activation(out=interp[:], in_=d[:], func=Act.Relu, bias=eps1[:, 0:1], scale=-1.0)
                outsb = finp.tile([P, 2], f32)
                sgn = finp.tile([P, CHUNKS], f32)
                nc.scalar.activation(
                    out=sgn[:], in_=interp[:], func=Act.Sign, bias=0.0, scale=1.0,
                    accum_out=outsb[:, 1:2],
                )
                sq = finp.tile([P, CHUNKS], f32)
                nc.scalar.square(sq[:], interp[:])
                cube = finp.tile([P, CHUNKS], f32)
                nc.vector.tensor_tensor(out=cube[:], in0=sq[:], in1=interp[:], op=Alu.mult)
                nc.vector.tensor_reduce(out=outsb[:, 0:1], in_=cube[:], axis=X, op=Alu.add)
                nc.sync.dma_start(out=out[:], in_=outsb[:])

    nc.compile()
    return nc


def _get_nc():
    if "nc" not in _CACHE:
        _CACHE["nc"] = _build_nc()
    return _CACHE["nc"]


def _make_in_maps(pred_pos, obstacle_pos, obstacle_faces):
    pred_pos = np.ascontiguousarray(np.asarray(pred_pos, dtype=np.float32))
    obstacle_pos = np.ascontiguousarray(np.asarray(obstacle_pos, dtype=np.float32))
    faces = np.ascontiguousarray(np.asarray(obstacle_faces).astype(np.int32))
    in_maps = []
    for c in range(NCORES):
        b, half = c // SPLIT, c % SPLIT
        pr = np.ascontiguousarray(pred_pos[b, half * NS:(half + 1) * NS])
        in_maps.append({
            "pred": pr,
            "predT": np.ascontiguousarray(pr.T),
            "opos": obstacle_pos[b],
            "faces": faces[b],
        })
    return in_maps


def kernel(pred_pos, obstacle_pos, obstacle_faces):
    from concourse.bass_utils import run_bass_kernel_spmd

    nc = _get_nc()
    in_maps = _make_in_maps(pred_pos, obstacle_pos, obstacle_faces)
    res = run_bass_kernel_spmd(nc, in_maps, core_ids=list(range(NCORES)))
    outs = np.stack([r["out"] for r in res.results])  # [8, 128, 2]
    loss_sum = float(outs[:, :, 0].astype(np.float64).sum())
    cnt_sum = float(outs[:, :, 1].astype(np.float64).sum())
    loss = np.float32(loss_sum / B * WEIGHT)
    perc = np.float32(cnt_sum / (B * N))
    return loss, perc

